# revision 9
# baseline (speedup 1.0000x reference)
"""Causal attention kernel for Trainium2 (8 NeuronCores, SPMD over heads).

Problem: B=4, H=16, S=2048, D=64, fp32.
  scores = Q @ K^T / sqrt(64); causal mask; softmax (global-max shift in the
  reference cancels exactly); out = attn @ V.

Distribution: B*H = 64 heads -> 8 heads per core, embarrassingly parallel.

Per-core algorithm (per head, two q-passes of 1024):
  - Q^T and K^T are duplicated into both partition halves (done host-side,
    uploaded pre-duplicated) so every matmul contracts over the full 128
    partitions: uniform 128x128 PE tile mode. The duplicated contraction
    computes 2*(Q.K); the 2x is folded into the exp scale/coefficients.
  - exp is split across two engines running concurrently: ScalarE (exact
    exp, scale=1/8, plus a constant bias matching the DVE path's systematic
    relative bias) and the DVE via a custom 8-stage op:
    p = ((c0*z + c1)*z + c2)^16 == e^(z/8)*(1+eps), eps nearly constant,
    cancelling in the softmax ratio. Tile assignment balances the engines.
  - Causal diagonal block: GpSimd multiply by a triangular keep-mask.
  - PV: one 128-contraction matmul chain per k-tile into per-512-column
    PSUM accumulator chunks; [V|ones] gives the softmax denominator in
    row 64 for free. Each acc chunk carries a precise stop flag (last
    k-tile that writes it), so its evacuation+DMA is emitted as soon as
    that chain ends -- evac work spreads through the pass instead of
    bursting at the seam.
  - Evacuation: ScalarE copies chunk 0, DVE chunk 1 (fp32 PSUM -> fp16
    SBUF), each DMA'd out as soon as it lands; the host does the final
    divide-by-rowsum and transpose.
  - Startup: cbeta + the first K chunk issue on the Scalar DGE queue in
    parallel with the Sync queue (Q first chunk + the rest), so the first
    matmul fires ~5us earlier than a single serial queue.
"""

import math
import os
import sys

import numpy as np

if "/opt/trn_rl_repo" not in sys.path:
    sys.path.insert(0, "/opt/trn_rl_repo")

B, H, S, D = 4, 16, 2048, 64
N_CORES = 8
HEADS_PER_CORE = (B * H) // N_CORES  # 8
PASS_Q = 1024  # q-columns per pass
CHUNK = 512  # PSUM bank boundary for fp32 outputs

# DVE exp: p = (C0*z + C1)*z + C2, squared 4x, z the duplicated-contraction
# score 2*(Q.K) (exp arg z/16).  ScalarE path: exp(z*0.0625 + BETA).
# Jointly optimized so both paths agree through the softmax ratio.
EXP_C0 = 3.4436267949839664e-05 / 4.0
EXP_C1 = 7.770817159682695e-03 / 2.0
EXP_C2 = 0.9999542988018534
EXP_BETA = -8.692886851909931e-04

_EXP_OP = [None]


def _register_exp_op():
    if _EXP_OP[0] is not None:
        return _EXP_OP[0]
    import concourse.dve_ops as dve_ops
    from concourse.dve_ops import DveOp
    from concourse.dve_spec import C0, C1, C2, Spec, Src0, sq

    def _ref(in0, in1, s0, s1, imm2):
        p = ((in0.astype(np.float32) * s0 + s1) * in0 + imm2).astype(np.float32)
        for _ in range(4):
            p = (p * p).astype(np.float32)
        return p

    op = DveOp(
        "EXP_PK16_ANT",
        Spec(body=sq(sq(sq(sq((Src0 * C0 + C1) * Src0 + C2)))), reference=_ref),
        subdim=False,
        uops_sha={"v3": "b9028a2770b985b4", "v4": "8a0143ec7033f2f1"},
    )
    if op.name not in dve_ops._SUB_OPCODE_FOR_NAME:
        dve_ops.OPS.append(op)
        dve_ops._SUB_OPCODE_FOR_NAME[op.name] = max(
            dve_ops._SUB_OPCODE_FOR_NAME.values()
        ) + 1
        dve_ops.CUSTOM_DVE_SPECS[op.name] = op.spec
    _EXP_OP[0] = op
    return op


def _chunks(lo, hi):
    """Split [lo, hi) at absolute multiples of CHUNK (PSUM bank boundaries)."""
    out = []
    c = lo
    while c < hi:
        w = min(hi, (c // CHUNK + 1) * CHUNK) - c
        out.append((c, w))
        c += w
    return out


def build_attention(tc, outs, ins, n_heads=HEADS_PER_CORE, s=S, pass_q=PASS_Q):
    import concourse.bass as bass
    import concourse.mybir as mybir

    exp_op = _register_exp_op()

    nc = tc.nc
    f32 = mybir.dt.float32
    f16 = mybir.dt.float16
    Exp = mybir.ActivationFunctionType.Exp

    qt_d, kt_d, v_d = ins["qt"], ins["kt"], ins["v"]
    tri_d = ins["ctri"]
    ot_d = outs["ot"]

    n_ktiles = s // 128
    n_pass = s // pass_q
    ktiles_per_pass = pass_q // 128
    n_chunks = pass_q // CHUNK  # acc chunks per pass

    with (
        tc.tile_pool(name="consts", bufs=1) as cpool,
        tc.tile_pool(name="qpool", bufs=3) as qpool,
        tc.tile_pool(name="kpool", bufs=3) as kpool,
        tc.tile_pool(name="vpool", bufs=3) as vpool,
        tc.tile_pool(name="atpool", bufs=12) as atpool,
        tc.tile_pool(name="osbpool", bufs=4) as osbpool,
        tc.tile_pool(name="scpool", bufs=3, space="PSUM") as scpool,
        tc.tile_pool(name="accpool", bufs=2, space="PSUM") as accApool,
    ):
        # --- input loads -------------------------------------------------
        qts, kts, vxs = {}, {}, {}
        c_beta = cpool.tile([128, 1], f32, tag="cbeta")
        c_tri = cpool.tile([128, 128], f16, tag="ctri")
        warm = cpool.tile([128, 1], f16, tag="warm")

        def emit_loads(h, startup=False):
            if h >= n_heads:
                return
            hs = s // 2
            qt2 = qpool.tile([128, s], f16, name="qt2", tag="qt")
            kt2 = kpool.tile([128, s], f16, name="kt2", tag="kt")
            vx = vpool.tile([128, n_ktiles * 65], f16, name="vx", tag="vx")
            qts[h], kts[h], vxs[h] = qt2, kt2, vx
            vx_v = vx.rearrange("p (t c) -> p t c", c=65)
            v_src = v_d[h].rearrange("(t p) d -> p t d", p=128)
            ht = n_ktiles // 2
            if startup:
                # fine-grained first bites split across both DGE queues in
                # need-order so the first QK matmuls fire as early as possible
                nc.sync.dma_start(kt2[:, 0:128], kt_d[h, :, 0:128])
                nc.scalar.dma_start(qt2[:, 0:512], qt_d[h, :, 0:512])
                nc.sync.dma_start(qt2[:, 512:hs], qt_d[h, :, 512:hs])
                nc.sync.dma_start(kt2[:, 128:hs], kt_d[h, :, 128:hs])
            else:
                nc.sync.dma_start(kt2[:, 0:hs], kt_d[h, :, 0:hs])
                nc.sync.dma_start(qt2[:, 0:hs], qt_d[h, :, 0:hs])
            nc.sync.dma_start(vx_v[:, 0:ht], v_src[:, 0:ht])
            nc.sync.dma_start(kt2[:, hs:], kt_d[h, :, hs:])
            nc.sync.dma_start(qt2[:, hs:], qt_d[h, :, hs:])
            nc.sync.dma_start(vx_v[:, ht:], v_src[:, ht:])

        emit_loads(0, startup=True)
        # consts follow the first bites on the Scalar DGE queue; the warm
        # dummy exp pulls ACT_TABLE_LOAD into the startup DMA window
        nc.scalar.dma_start(c_beta[:], ins["cbeta"][:])
        nc.scalar.dma_start(c_tri[:], tri_d[:])
        nc.scalar.activation(warm[:], c_beta[:], Exp, scale=0.0)
        # PE warmup: dummy matmuls on a zeroed tile run during the startup
        # DMA window so the p-state ramp and instruction fetch complete
        # before the first real matmul's inputs land
        wsrc = cpool.tile([128, 512], f16, tag="wsrc")
        nc.gpsimd.memset(wsrc[:], 0.0)
        for wi in range(4):
            wsc = scpool.tile([128, pass_q], f32, tag="sc", name=f"wsc_{wi}")
            nc.tensor.matmul(
                wsc[:, 0:512], wsrc[:, 0:128], wsrc[:],
                start=True, stop=True, skip_group_check=True,
            )
        emit_loads(1)

        # Cross-pass software pipeline: the last two PV pairs (and the final
        # acc chunk's evacuation) of pass p are emitted right after the first
        # QK pair of the next pass, so the PE never drains at pass/head
        # boundaries.
        pending_tail = [None]

        def _flush_tail():
            if pending_tail[0] is not None:
                pending_tail[0]()
                pending_tail[0] = None

        for h in range(n_heads):
            qt2, kt2, vx = qts[h], kts[h], vxs[h]
            vx_v = vx.rearrange("p (t c) -> p t c", c=65)
            kt2_v = kt2.rearrange("p (t c) -> p t c", c=128)

            for p in range(n_pass):
                if h + 2 <= n_heads - 1 and p == 1:
                    emit_loads(h + 2)
                q0 = p * pass_q
                kmax = (p + 1) * ktiles_per_pass
                # per-512-chunk accumulators; chunk c's chain ends at k-tile
                # last_w[c], whose PV carries the stop flag -> early evac
                accs = [
                    accApool.tile([65, CHUNK], f32, name="acc", tag="acc")
                    for c in range(n_chunks)
                ]
                last_w = [
                    min(kmax, (q0 + (c + 1) * CHUNK) // 128) - 1
                    for c in range(n_chunks)
                ]
                pv_queue = []

                def _evac(c, h=h, p=p, q0=q0, accs=accs):
                    """Evacuate acc chunk c (fp32 PSUM -> fp16 SBUF) + DMA."""
                    osb = osbpool.tile([65, CHUNK], f16, name="osb", tag="osb")
                    a0 = q0 + c * CHUNK
                    if c % 2 == 0:
                        nc.scalar.copy(osb[:], accs[c][:])
                    else:
                        nc.vector.tensor_copy(osb[:], accs[c][:])
                    nc.sync.dma_start(ot_d[h, :, a0 : a0 + CHUNK], osb[:])

                def _evac_piece(i, lo, hi, h=h, p=p, q0=q0, accs=accs):
                    """Final-pass fine-grained evac of acc cols [lo,hi) as the
                    per-column accumulation chains end; alternate engines and
                    DMA queues so the last piece's path is minimal."""
                    osb = osbpool.tile([65, CHUNK], f16, name="osb", tag="osb")
                    c, w = lo // CHUNK, hi - lo
                    cc = lo % CHUNK
                    if i % 2 == 0:
                        nc.scalar.copy(osb[:, 0:w], accs[c][:, cc : cc + w])
                    else:
                        nc.vector.tensor_copy(osb[:, 0:w], accs[c][:, cc : cc + w])
                    dma_eng = nc.sync if i % 2 == 0 else nc.scalar
                    dma_eng.dma_start(ot_d[h, :, q0 + lo : q0 + hi], osb[:, 0:w])

                def _emit_pv(entries, accs=accs, last_w=last_w, q0=q0, vx_v=vx_v,
                             evac=_evac, evacp=_evac_piece, kmax=kmax, final=False):
                    for (k, at, qlo) in entries:
                        for (c, w) in _chunks(qlo - q0, pass_q):
                            co = c - (qlo - q0)
                            ci, cc = c // CHUNK, c % CHUNK
                            nc.tensor.matmul(
                                accs[ci][0:65, cc : cc + w],
                                vx_v[:, k, :],
                                at[:, co : co + w],
                                start=(k == 0),
                                stop=(k == last_w[ci]),
                                skip_group_check=True,
                            )
                        if final and k >= kmax - 4:
                            lo = max(CHUNK, 128 * k - q0)
                            hi = min(pass_q, 128 * (k + 1) - q0)
                            if hi > lo:
                                evacp(k, lo, hi)
                        for ci in range(len(accs)):
                            if k == last_w[ci] and ci < len(accs) - 1:
                                evac(ci)

                # exp engine assignment: balance ScalarE (0.833ns/col+185)
                # vs DVE (1.04ns/col+125): DVE takes odd k-tiles except the
                # two largest odd spans per pass go to ScalarE.
                for kp in range(0, kmax, 2):
                    pair = [k for k in (kp, kp + 1) if k < kmax]
                    scs, spans, qlos = {}, {}, {}
                    for k in pair:
                        qlos[k] = max(q0, 128 * k)
                        spans[k] = q0 + pass_q - qlos[k]
                        scs[k] = scpool.tile(
                            [128, pass_q], f32, tag="sc", name=f"sc_{h}_{p}_{k}"
                        )
                    for k in pair:
                        for (c, w) in _chunks(0, spans[k]):
                            nc.tensor.matmul(
                                scs[k][:, c : c + w],
                                kt2_v[:, k],
                                qt2[:, qlos[k] + c : qlos[k] + c + w],
                                start=True,
                                stop=True,
                                skip_group_check=True,
                            )
                    if kp == 0:
                        _flush_tail()
                    cur = []
                    for k in pair:
                        span, qlo = spans[k], qlos[k]
                        at = atpool.tile([128, pass_q], f16)
                        if k % 2 == 0:
                            nc.scalar.activation(
                                at[:, 0:span], scs[k][:, 0:span], Exp,
                                bias=c_beta[:, 0:1], scale=0.0625,
                            )
                        else:
                            nc.vector._custom_dve(
                                exp_op,
                                out=at[:, 0:span],
                                in0=scs[k][:, 0:span],
                                s0=EXP_C0, s1=EXP_C1, imm2=EXP_C2,
                            )
                        if 128 * k >= q0:
                            # zero the masked upper part of the diagonal block
                            nc.gpsimd.tensor_mul(at[:, 0:128], at[:, 0:128], c_tri[:])
                        cur.append((k, at, qlo))
                    pv_queue.append(cur)
                    if len(pv_queue) > 3:
                        _emit_pv(pv_queue.pop(0))
                leftovers = list(pv_queue)
                is_final = h == n_heads - 1 and p == n_pass - 1

                def _tail(leftovers=leftovers, emit=_emit_pv, evac=_evac,
                          nch=n_chunks, final=is_final):
                    for entries in leftovers:
                        emit(entries, final=final)
                    if not final:
                        evac(nch - 1)

                if is_final:
                    _tail()
                else:
                    pending_tail[0] = _tail
        _flush_tail()


def _make_consts():
    kk, qq = np.meshgrid(np.arange(128), np.arange(128), indexing="ij")
    tri = (kk <= qq).astype(np.float16)  # keep-mask for the diagonal block
    return tri


_NC_CACHE = {}


def _build_nc(n_heads=HEADS_PER_CORE, s=S, pass_q=PASS_Q):
    key = (n_heads, s, pass_q)
    if key in _NC_CACHE:
        return _NC_CACHE[key]
    import concourse.tile as tile
    from concourse import bacc, mybir

    nc = bacc.Bacc(
        "TRN2", target_bir_lowering=False, debug=False, enable_asserts=False
    )
    f32 = mybir.dt.float32
    f16 = mybir.dt.float16
    ins = {
        "qt": nc.dram_tensor("qt", [n_heads, 128, s], f16, kind="ExternalInput").ap(),
        "kt": nc.dram_tensor("kt", [n_heads, 128, s], f16, kind="ExternalInput").ap(),
        "v": nc.dram_tensor("v", [n_heads, s, D + 1], f16, kind="ExternalInput").ap(),
        "ctri": nc.dram_tensor("ctri", [128, 128], f16, kind="ExternalInput").ap(),
        "cbeta": nc.dram_tensor("cbeta", [128, 1], f32, kind="ExternalInput").ap(),
    }
    outs = {
        "ot": nc.dram_tensor("ot", [n_heads, 65, s], f16, kind="ExternalOutput").ap(),
    }
    with tile.TileContext(nc) as tc:
        build_attention(tc, outs, ins, n_heads=n_heads, s=s, pass_q=pass_q)
    nc.compile()
    _NC_CACHE[key] = nc
    return nc


def kernel(Q, K, V, mask, trace=False):
    """Full-input entry point: shards over 8 NeuronCores, returns full output."""
    from concourse.bass_utils import run_bass_kernel_spmd

    nc = _build_nc()
    tri = _make_consts()

    Qf = np.ascontiguousarray(
        Q.reshape(B * H, S, D).transpose(0, 2, 1), dtype=np.float16
    )
    Kf = np.ascontiguousarray(
        K.reshape(B * H, S, D).transpose(0, 2, 1), dtype=np.float16
    )
    # duplicate into both partition halves host-side (one DMA instr per load)
    Qf = np.ascontiguousarray(np.concatenate([Qf, Qf], axis=1))
    Kf = np.ascontiguousarray(np.concatenate([Kf, Kf], axis=1))
    Vf = np.concatenate(
        [
            V.reshape(B * H, S, D).astype(np.float16),
            np.ones((B * H, S, 1), dtype=np.float16),
        ],
        axis=-1,
    )

    in_maps = []
    for c in range(N_CORES):
        sl = slice(c * HEADS_PER_CORE, (c + 1) * HEADS_PER_CORE)
        in_maps.append(
            {
                "qt": Qf[sl],
                "kt": Kf[sl],
                "v": Vf[sl],
                "ctri": tri,
                "cbeta": np.full((128, 1), EXP_BETA, dtype=np.float32),
            }
        )

    res = run_bass_kernel_spmd(nc, in_maps, core_ids=list(range(N_CORES)), trace=trace)
    ot = np.concatenate(
        [res.results[c]["ot"].astype(np.float32) for c in range(N_CORES)], axis=0
    )
    # ot: [B*H, 65, S] -- rows 0..63 are out^T columns, row 64 the rowsum.
    out = (ot[:, :64, :] / ot[:, 64:65, :]).transpose(0, 2, 1)
    out = out.reshape(B, H, S, D)
    kernel.last_results = res
    return np.ascontiguousarray(out, dtype=np.float32)


# revision 10
# speedup vs baseline: 1.0004x; 1.0004x over previous
"""Causal attention kernel for Trainium2 (8 NeuronCores, SPMD over heads).

Problem: B=4, H=16, S=2048, D=64, fp32.
  scores = Q @ K^T / sqrt(64); causal mask; softmax (global-max shift in the
  reference cancels exactly); out = attn @ V.

Distribution: B*H = 64 heads -> 8 heads per core, embarrassingly parallel.

Per-core algorithm (per head, two q-passes of 1024):
  - Q^T and K^T are duplicated into both partition halves (done host-side,
    uploaded pre-duplicated) so every matmul contracts over the full 128
    partitions: uniform 128x128 PE tile mode. The duplicated contraction
    computes 2*(Q.K); the 2x is folded into the exp scale/coefficients.
  - exp is split across two engines running concurrently: ScalarE (exact
    exp, scale=1/8, plus a constant bias matching the DVE path's systematic
    relative bias) and the DVE via a custom 8-stage op:
    p = ((c0*z + c1)*z + c2)^16 == e^(z/8)*(1+eps), eps nearly constant,
    cancelling in the softmax ratio. Tile assignment balances the engines.
  - Causal diagonal block: GpSimd multiply by a triangular keep-mask.
  - PV: one 128-contraction matmul chain per k-tile into per-512-column
    PSUM accumulator chunks; [V|ones] gives the softmax denominator in
    row 64 for free. Each acc chunk carries a precise stop flag (last
    k-tile that writes it), so its evacuation+DMA is emitted as soon as
    that chain ends -- evac work spreads through the pass instead of
    bursting at the seam.
  - Evacuation: ScalarE copies chunk 0, DVE chunk 1 (fp32 PSUM -> fp16
    SBUF), each DMA'd out as soon as it lands; the host does the final
    divide-by-rowsum and transpose.
  - Startup: cbeta + the first K chunk issue on the Scalar DGE queue in
    parallel with the Sync queue (Q first chunk + the rest), so the first
    matmul fires ~5us earlier than a single serial queue.
"""

import math
import os
import sys

import numpy as np

if "/opt/trn_rl_repo" not in sys.path:
    sys.path.insert(0, "/opt/trn_rl_repo")

B, H, S, D = 4, 16, 2048, 64
N_CORES = 8
HEADS_PER_CORE = (B * H) // N_CORES  # 8
PASS_Q = 1024  # q-columns per pass
CHUNK = 512  # PSUM bank boundary for fp32 outputs

# DVE exp: p = (C0*z + C1)*z + C2, squared 4x, z the duplicated-contraction
# score 2*(Q.K) (exp arg z/16).  ScalarE path: exp(z*0.0625 + BETA).
# Jointly optimized so both paths agree through the softmax ratio.
EXP_C0 = 3.4436267949839664e-05 / 4.0
EXP_C1 = 7.770817159682695e-03 / 2.0
EXP_C2 = 0.9999542988018534
EXP_BETA = -8.692886851909931e-04

_EXP_OP = [None]


def _register_exp_op():
    if _EXP_OP[0] is not None:
        return _EXP_OP[0]
    import concourse.dve_ops as dve_ops
    from concourse.dve_ops import DveOp
    from concourse.dve_spec import C0, C1, C2, Spec, Src0, sq

    def _ref(in0, in1, s0, s1, imm2):
        p = ((in0.astype(np.float32) * s0 + s1) * in0 + imm2).astype(np.float32)
        for _ in range(4):
            p = (p * p).astype(np.float32)
        return p

    op = DveOp(
        "EXP_PK16_ANT",
        Spec(body=sq(sq(sq(sq((Src0 * C0 + C1) * Src0 + C2)))), reference=_ref),
        subdim=False,
        uops_sha={"v3": "b9028a2770b985b4", "v4": "8a0143ec7033f2f1"},
    )
    if op.name not in dve_ops._SUB_OPCODE_FOR_NAME:
        dve_ops.OPS.append(op)
        dve_ops._SUB_OPCODE_FOR_NAME[op.name] = max(
            dve_ops._SUB_OPCODE_FOR_NAME.values()
        ) + 1
        dve_ops.CUSTOM_DVE_SPECS[op.name] = op.spec
    _EXP_OP[0] = op
    return op


def _chunks(lo, hi):
    """Split [lo, hi) at absolute multiples of CHUNK (PSUM bank boundaries)."""
    out = []
    c = lo
    while c < hi:
        w = min(hi, (c // CHUNK + 1) * CHUNK) - c
        out.append((c, w))
        c += w
    return out


def build_attention(tc, outs, ins, n_heads=HEADS_PER_CORE, s=S, pass_q=PASS_Q):
    import concourse.bass as bass
    import concourse.mybir as mybir

    exp_op = _register_exp_op()

    nc = tc.nc
    f32 = mybir.dt.float32
    f16 = mybir.dt.float16
    Exp = mybir.ActivationFunctionType.Exp

    qt_d, kt_d, v_d = ins["qt"], ins["kt"], ins["v"]
    tri_d = ins["ctri"]
    ot_d = outs["ot"]

    n_ktiles = s // 128
    n_pass = s // pass_q
    ktiles_per_pass = pass_q // 128
    n_chunks = pass_q // CHUNK  # acc chunks per pass

    with (
        tc.tile_pool(name="consts", bufs=1) as cpool,
        tc.tile_pool(name="qpool", bufs=3) as qpool,
        tc.tile_pool(name="kpool", bufs=3) as kpool,
        tc.tile_pool(name="vpool", bufs=3) as vpool,
        tc.tile_pool(name="atpool", bufs=12) as atpool,
        tc.tile_pool(name="osbpool", bufs=4) as osbpool,
        tc.tile_pool(name="scpool", bufs=3, space="PSUM") as scpool,
        tc.tile_pool(name="accpool", bufs=2, space="PSUM") as accApool,
    ):
        # --- input loads -------------------------------------------------
        qts, kts, vxs = {}, {}, {}
        c_beta = cpool.tile([128, 1], f32, tag="cbeta")
        c_tri = cpool.tile([128, 128], f16, tag="ctri")
        warm = cpool.tile([128, 1], f16, tag="warm")

        def emit_loads(h, startup=False):
            if h >= n_heads:
                return
            hs = s // 2
            qt2 = qpool.tile([128, s], f16, name="qt2", tag="qt")
            kt2 = kpool.tile([128, s], f16, name="kt2", tag="kt")
            vx = vpool.tile([128, n_ktiles * 65], f16, name="vx", tag="vx")
            qts[h], kts[h], vxs[h] = qt2, kt2, vx
            vx_v = vx.rearrange("p (t c) -> p t c", c=65)
            v_src = v_d[h].rearrange("(t p) d -> p t d", p=128)
            ht = n_ktiles // 2
            if startup:
                # fine-grained first bites split across both DGE queues in
                # need-order so the first QK matmuls fire as early as possible
                nc.sync.dma_start(kt2[:, 0:512], kt_d[h, :, 0:512])
                nc.scalar.dma_start(qt2[:, 0:512], qt_d[h, :, 0:512])
                nc.sync.dma_start(qt2[:, 512:hs], qt_d[h, :, 512:hs])
                nc.sync.dma_start(kt2[:, 512:hs], kt_d[h, :, 512:hs])
            else:
                nc.sync.dma_start(kt2[:, 0:hs], kt_d[h, :, 0:hs])
                nc.sync.dma_start(qt2[:, 0:hs], qt_d[h, :, 0:hs])
            nc.sync.dma_start(vx_v[:, 0:ht], v_src[:, 0:ht])
            nc.sync.dma_start(kt2[:, hs:], kt_d[h, :, hs:])
            nc.sync.dma_start(qt2[:, hs:], qt_d[h, :, hs:])
            nc.sync.dma_start(vx_v[:, ht:], v_src[:, ht:])

        emit_loads(0, startup=True)
        # consts follow the first bites on the Scalar DGE queue; the warm
        # dummy exp pulls ACT_TABLE_LOAD into the startup DMA window
        nc.scalar.dma_start(c_beta[:], ins["cbeta"][:])
        nc.scalar.dma_start(c_tri[:], tri_d[:])
        nc.scalar.activation(warm[:], c_beta[:], Exp, scale=0.0)
        # PE warmup: dummy matmuls on a zeroed tile run during the startup
        # DMA window so the p-state ramp and instruction fetch complete
        # before the first real matmul's inputs land
        wsrc = cpool.tile([128, 512], f16, tag="wsrc")
        nc.gpsimd.memset(wsrc[:], 0.0)
        for wi in range(5):
            wsc = scpool.tile([128, pass_q], f32, tag="sc", name=f"wsc_{wi}")
            nc.tensor.matmul(
                wsc[:, 0:512], wsrc[:, 0:128], wsrc[:],
                start=True, stop=True, skip_group_check=True,
            )
        emit_loads(1)

        # Cross-pass software pipeline: the last two PV pairs (and the final
        # acc chunk's evacuation) of pass p are emitted right after the first
        # QK pair of the next pass, so the PE never drains at pass/head
        # boundaries.
        pending_tail = [None]

        def _flush_tail():
            if pending_tail[0] is not None:
                pending_tail[0]()
                pending_tail[0] = None

        for h in range(n_heads):
            qt2, kt2, vx = qts[h], kts[h], vxs[h]
            vx_v = vx.rearrange("p (t c) -> p t c", c=65)
            kt2_v = kt2.rearrange("p (t c) -> p t c", c=128)

            for p in range(n_pass):
                if h + 2 <= n_heads - 1 and p == 1:
                    emit_loads(h + 2)
                q0 = p * pass_q
                kmax = (p + 1) * ktiles_per_pass
                # per-512-chunk accumulators; chunk c's chain ends at k-tile
                # last_w[c], whose PV carries the stop flag -> early evac
                accs = [
                    accApool.tile([65, CHUNK], f32, name="acc", tag="acc")
                    for c in range(n_chunks)
                ]
                last_w = [
                    min(kmax, (q0 + (c + 1) * CHUNK) // 128) - 1
                    for c in range(n_chunks)
                ]
                pv_queue = []

                def _evac(c, h=h, p=p, q0=q0, accs=accs):
                    """Evacuate acc chunk c (fp32 PSUM -> fp16 SBUF) + DMA."""
                    osb = osbpool.tile([65, CHUNK], f16, name="osb", tag="osb")
                    a0 = q0 + c * CHUNK
                    if c % 2 == 0:
                        nc.scalar.copy(osb[:], accs[c][:])
                    else:
                        nc.vector.tensor_copy(osb[:], accs[c][:])
                    nc.sync.dma_start(ot_d[h, :, a0 : a0 + CHUNK], osb[:])

                def _evac_piece(i, lo, hi, h=h, p=p, q0=q0, accs=accs):
                    """Final-pass fine-grained evac of acc cols [lo,hi) as the
                    per-column accumulation chains end; alternate engines and
                    DMA queues so the last piece's path is minimal."""
                    osb = osbpool.tile([65, CHUNK], f16, name="osb", tag="osb")
                    c, w = lo // CHUNK, hi - lo
                    cc = lo % CHUNK
                    if i % 2 == 0:
                        nc.scalar.copy(osb[:, 0:w], accs[c][:, cc : cc + w])
                    else:
                        nc.vector.tensor_copy(osb[:, 0:w], accs[c][:, cc : cc + w])
                    dma_eng = nc.sync if i % 2 == 0 else nc.scalar
                    dma_eng.dma_start(ot_d[h, :, q0 + lo : q0 + hi], osb[:, 0:w])

                def _emit_pv(entries, accs=accs, last_w=last_w, q0=q0, vx_v=vx_v,
                             evac=_evac, evacp=_evac_piece, kmax=kmax, final=False):
                    for (k, at, qlo) in entries:
                        for (c, w) in _chunks(qlo - q0, pass_q):
                            co = c - (qlo - q0)
                            ci, cc = c // CHUNK, c % CHUNK
                            nc.tensor.matmul(
                                accs[ci][0:65, cc : cc + w],
                                vx_v[:, k, :],
                                at[:, co : co + w],
                                start=(k == 0),
                                stop=(k == last_w[ci]),
                                skip_group_check=True,
                            )
                        if final and k >= kmax - 4:
                            lo = max(CHUNK, 128 * k - q0)
                            hi = min(pass_q, 128 * (k + 1) - q0)
                            if hi > lo:
                                evacp(k, lo, hi)
                        for ci in range(len(accs)):
                            if k == last_w[ci] and ci < len(accs) - 1:
                                evac(ci)

                # exp engine assignment: balance ScalarE (0.833ns/col+185)
                # vs DVE (1.04ns/col+125): DVE takes odd k-tiles except the
                # two largest odd spans per pass go to ScalarE.
                for kp in range(0, kmax, 2):
                    pair = [k for k in (kp, kp + 1) if k < kmax]
                    scs, spans, qlos = {}, {}, {}
                    for k in pair:
                        qlos[k] = max(q0, 128 * k)
                        spans[k] = q0 + pass_q - qlos[k]
                        scs[k] = scpool.tile(
                            [128, pass_q], f32, tag="sc", name=f"sc_{h}_{p}_{k}"
                        )
                    for k in pair:
                        for (c, w) in _chunks(0, spans[k]):
                            nc.tensor.matmul(
                                scs[k][:, c : c + w],
                                kt2_v[:, k],
                                qt2[:, qlos[k] + c : qlos[k] + c + w],
                                start=True,
                                stop=True,
                                skip_group_check=True,
                            )
                    if kp == 0:
                        _flush_tail()
                    cur = []
                    for k in pair:
                        span, qlo = spans[k], qlos[k]
                        at = atpool.tile([128, pass_q], f16)
                        if k % 2 == 0:
                            nc.scalar.activation(
                                at[:, 0:span], scs[k][:, 0:span], Exp,
                                bias=c_beta[:, 0:1], scale=0.0625,
                            )
                        else:
                            nc.vector._custom_dve(
                                exp_op,
                                out=at[:, 0:span],
                                in0=scs[k][:, 0:span],
                                s0=EXP_C0, s1=EXP_C1, imm2=EXP_C2,
                            )
                        if 128 * k >= q0:
                            # zero the masked upper part of the diagonal block
                            nc.gpsimd.tensor_mul(at[:, 0:128], at[:, 0:128], c_tri[:])
                        cur.append((k, at, qlo))
                    pv_queue.append(cur)
                    if len(pv_queue) > 3:
                        _emit_pv(pv_queue.pop(0))
                leftovers = list(pv_queue)
                is_final = h == n_heads - 1 and p == n_pass - 1

                def _tail(leftovers=leftovers, emit=_emit_pv, evac=_evac,
                          nch=n_chunks, final=is_final):
                    for entries in leftovers:
                        emit(entries, final=final)
                    if not final:
                        evac(nch - 1)

                if is_final:
                    _tail()
                else:
                    pending_tail[0] = _tail
        _flush_tail()


def _make_consts():
    kk, qq = np.meshgrid(np.arange(128), np.arange(128), indexing="ij")
    tri = (kk <= qq).astype(np.float16)  # keep-mask for the diagonal block
    return tri


_NC_CACHE = {}


def _build_nc(n_heads=HEADS_PER_CORE, s=S, pass_q=PASS_Q):
    key = (n_heads, s, pass_q)
    if key in _NC_CACHE:
        return _NC_CACHE[key]
    import concourse.tile as tile
    from concourse import bacc, mybir

    nc = bacc.Bacc(
        "TRN2", target_bir_lowering=False, debug=False, enable_asserts=False
    )
    f32 = mybir.dt.float32
    f16 = mybir.dt.float16
    ins = {
        "qt": nc.dram_tensor("qt", [n_heads, 128, s], f16, kind="ExternalInput").ap(),
        "kt": nc.dram_tensor("kt", [n_heads, 128, s], f16, kind="ExternalInput").ap(),
        "v": nc.dram_tensor("v", [n_heads, s, D + 1], f16, kind="ExternalInput").ap(),
        "ctri": nc.dram_tensor("ctri", [128, 128], f16, kind="ExternalInput").ap(),
        "cbeta": nc.dram_tensor("cbeta", [128, 1], f32, kind="ExternalInput").ap(),
    }
    outs = {
        "ot": nc.dram_tensor("ot", [n_heads, 65, s], f16, kind="ExternalOutput").ap(),
    }
    with tile.TileContext(nc) as tc:
        build_attention(tc, outs, ins, n_heads=n_heads, s=s, pass_q=pass_q)
    nc.compile()
    _NC_CACHE[key] = nc
    return nc


def kernel(Q, K, V, mask, trace=False):
    """Full-input entry point: shards over 8 NeuronCores, returns full output."""
    from concourse.bass_utils import run_bass_kernel_spmd

    nc = _build_nc()
    tri = _make_consts()

    Qf = np.ascontiguousarray(
        Q.reshape(B * H, S, D).transpose(0, 2, 1), dtype=np.float16
    )
    Kf = np.ascontiguousarray(
        K.reshape(B * H, S, D).transpose(0, 2, 1), dtype=np.float16
    )
    # duplicate into both partition halves host-side (one DMA instr per load)
    Qf = np.ascontiguousarray(np.concatenate([Qf, Qf], axis=1))
    Kf = np.ascontiguousarray(np.concatenate([Kf, Kf], axis=1))
    Vf = np.concatenate(
        [
            V.reshape(B * H, S, D).astype(np.float16),
            np.ones((B * H, S, 1), dtype=np.float16),
        ],
        axis=-1,
    )

    in_maps = []
    for c in range(N_CORES):
        sl = slice(c * HEADS_PER_CORE, (c + 1) * HEADS_PER_CORE)
        in_maps.append(
            {
                "qt": Qf[sl],
                "kt": Kf[sl],
                "v": Vf[sl],
                "ctri": tri,
                "cbeta": np.full((128, 1), EXP_BETA, dtype=np.float32),
            }
        )

    res = run_bass_kernel_spmd(nc, in_maps, core_ids=list(range(N_CORES)), trace=trace)
    ot = np.concatenate(
        [res.results[c]["ot"].astype(np.float32) for c in range(N_CORES)], axis=0
    )
    # ot: [B*H, 65, S] -- rows 0..63 are out^T columns, row 64 the rowsum.
    out = (ot[:, :64, :] / ot[:, 64:65, :]).transpose(0, 2, 1)
    out = out.reshape(B, H, S, D)
    kernel.last_results = res
    return np.ascontiguousarray(out, dtype=np.float32)


# revision 11
# speedup vs baseline: 1.0093x; 1.0089x over previous
"""Causal attention kernel for Trainium2 (8 NeuronCores, SPMD over heads).

Problem: B=4, H=16, S=2048, D=64, fp32.
  scores = Q @ K^T / sqrt(64); causal mask; softmax (global-max shift in the
  reference cancels exactly); out = attn @ V.

Distribution: B*H = 64 heads -> 8 heads per core, embarrassingly parallel.

Per-core algorithm (per head, two q-passes of 1024):
  - Q^T and K^T are duplicated into both partition halves (done host-side,
    uploaded pre-duplicated) so every matmul contracts over the full 128
    partitions: uniform 128x128 PE tile mode. The duplicated contraction
    computes 2*(Q.K); the 2x is folded into the exp scale/coefficients.
  - exp is split across two engines running concurrently: ScalarE (exact
    exp, scale=1/8, plus a constant bias matching the DVE path's systematic
    relative bias) and the DVE via a custom 8-stage op:
    p = ((c0*z + c1)*z + c2)^16 == e^(z/8)*(1+eps), eps nearly constant,
    cancelling in the softmax ratio. Tile assignment balances the engines.
  - Causal diagonal block: GpSimd multiply by a triangular keep-mask.
  - PV: one 128-contraction matmul chain per k-tile into per-512-column
    PSUM accumulator chunks; [V|ones] gives the softmax denominator in
    row 64 for free. Each acc chunk carries a precise stop flag (last
    k-tile that writes it), so its evacuation+DMA is emitted as soon as
    that chain ends -- evac work spreads through the pass instead of
    bursting at the seam.
  - Evacuation: ScalarE copies chunk 0, DVE chunk 1 (fp32 PSUM -> fp16
    SBUF), each DMA'd out as soon as it lands; the host does the final
    divide-by-rowsum and transpose.
  - Startup: cbeta + the first K chunk issue on the Scalar DGE queue in
    parallel with the Sync queue (Q first chunk + the rest), so the first
    matmul fires ~5us earlier than a single serial queue.
"""

import math
import os
import sys

import numpy as np

if "/opt/trn_rl_repo" not in sys.path:
    sys.path.insert(0, "/opt/trn_rl_repo")

B, H, S, D = 4, 16, 2048, 64
N_CORES = 8
HEADS_PER_CORE = (B * H) // N_CORES  # 8
PASS_Q = 1024  # q-columns per pass
CHUNK = 512  # PSUM bank boundary for fp32 outputs

# DVE exp: p = (C0*z + C1)*z + C2, squared 4x, z the duplicated-contraction
# score 2*(Q.K) (exp arg z/16).  ScalarE path: exp(z*0.0625 + BETA).
# Jointly optimized so both paths agree through the softmax ratio.
EXP_C0 = 3.4436267949839664e-05 / 4.0
EXP_C1 = 7.770817159682695e-03 / 2.0
EXP_C2 = 0.9999542988018534
EXP_BETA = -8.692886851909931e-04

_EXP_OP = [None]


def _register_exp_op():
    if _EXP_OP[0] is not None:
        return _EXP_OP[0]
    import concourse.dve_ops as dve_ops
    from concourse.dve_ops import DveOp
    from concourse.dve_spec import C0, C1, C2, Spec, Src0, sq

    def _ref(in0, in1, s0, s1, imm2):
        p = ((in0.astype(np.float32) * s0 + s1) * in0 + imm2).astype(np.float32)
        for _ in range(4):
            p = (p * p).astype(np.float32)
        return p

    op = DveOp(
        "EXP_PK16_ANT",
        Spec(body=sq(sq(sq(sq((Src0 * C0 + C1) * Src0 + C2)))), reference=_ref),
        subdim=False,
        uops_sha={"v3": "b9028a2770b985b4", "v4": "8a0143ec7033f2f1"},
    )
    if op.name not in dve_ops._SUB_OPCODE_FOR_NAME:
        dve_ops.OPS.append(op)
        dve_ops._SUB_OPCODE_FOR_NAME[op.name] = max(
            dve_ops._SUB_OPCODE_FOR_NAME.values()
        ) + 1
        dve_ops.CUSTOM_DVE_SPECS[op.name] = op.spec
    _EXP_OP[0] = op
    return op


def _chunks(lo, hi):
    """Split [lo, hi) at absolute multiples of CHUNK (PSUM bank boundaries)."""
    out = []
    c = lo
    while c < hi:
        w = min(hi, (c // CHUNK + 1) * CHUNK) - c
        out.append((c, w))
        c += w
    return out


def build_attention(tc, outs, ins, n_heads=HEADS_PER_CORE, s=S, pass_q=PASS_Q):
    import concourse.bass as bass
    import concourse.mybir as mybir

    exp_op = _register_exp_op()

    nc = tc.nc
    f32 = mybir.dt.float32
    f16 = mybir.dt.float16
    Exp = mybir.ActivationFunctionType.Exp

    qt_d, kt_d, v_d = ins["qt"], ins["kt"], ins["v"]
    tri_d = ins["ctri"]
    ot_d = outs["ot"]

    n_ktiles = s // 128
    n_pass = s // pass_q
    ktiles_per_pass = pass_q // 128
    n_chunks = pass_q // CHUNK  # acc chunks per pass

    with (
        tc.tile_pool(name="consts", bufs=1) as cpool,
        tc.tile_pool(name="qpool", bufs=3) as qpool,
        tc.tile_pool(name="kpool", bufs=3) as kpool,
        tc.tile_pool(name="vpool", bufs=3) as vpool,
        tc.tile_pool(name="atpool", bufs=12) as atpool,
        tc.tile_pool(name="osbpool", bufs=4) as osbpool,
        tc.tile_pool(name="scpool", bufs=3, space="PSUM") as scpool,
        tc.tile_pool(name="accpool", bufs=2, space="PSUM") as accApool,
    ):
        # --- input loads -------------------------------------------------
        qts, kts, vxs = {}, {}, {}
        c_beta = cpool.tile([128, 1], f32, tag="cbeta")
        c_tri = cpool.tile([128, 128], f16, tag="ctri")
        warm = cpool.tile([128, 1], f16, tag="warm")

        def emit_loads(h, startup=False):
            if h >= n_heads:
                return
            hs = s // 2
            qt2 = qpool.tile([128, s], f16, name="qt2", tag="qt")
            kt2 = kpool.tile([128, s], f16, name="kt2", tag="kt")
            vx = vpool.tile([128, n_ktiles * 65], f16, name="vx", tag="vx")
            qts[h], kts[h], vxs[h] = qt2, kt2, vx
            vx_v = vx.rearrange("p (t c) -> p t c", c=65)
            v_src = v_d[h].rearrange("(t p) d -> p t d", p=128)
            ht = n_ktiles // 2
            if startup:
                # fine-grained first bites split across both DGE queues in
                # need-order so the first QK matmuls fire as early as possible
                nc.sync.dma_start(kt2[:, 0:512], kt_d[h, :, 0:512])
                nc.scalar.dma_start(qt2[:, 0:512], qt_d[h, :, 0:512])
                nc.sync.dma_start(qt2[:, 512:hs], qt_d[h, :, 512:hs])
                nc.sync.dma_start(kt2[:, 512:hs], kt_d[h, :, 512:hs])
            else:
                nc.sync.dma_start(kt2[:, 0:hs], kt_d[h, :, 0:hs])
                nc.sync.dma_start(qt2[:, 0:hs], qt_d[h, :, 0:hs])
            nc.sync.dma_start(vx_v[:, 0:ht], v_src[:, 0:ht])
            nc.sync.dma_start(kt2[:, hs:], kt_d[h, :, hs:])
            nc.sync.dma_start(qt2[:, hs:], qt_d[h, :, hs:])
            nc.sync.dma_start(vx_v[:, ht:], v_src[:, ht:])

        emit_loads(0, startup=True)
        # consts follow the first bites on the Scalar DGE queue; the warm
        # dummy exp pulls ACT_TABLE_LOAD into the startup DMA window
        nc.scalar.dma_start(c_beta[:], ins["cbeta"][:])
        nc.scalar.dma_start(c_tri[:], tri_d[:])
        nc.scalar.activation(warm[:], c_beta[:], Exp, scale=0.0)
        # PE warmup: dummy matmuls on a zeroed tile run during the startup
        # DMA window so the p-state ramp and instruction fetch complete
        # before the first real matmul's inputs land
        wsrc = cpool.tile([128, 512], f16, tag="wsrc")
        nc.gpsimd.memset(wsrc[:], 0.0)
        for wi in range(6):
            wsc = scpool.tile([128, pass_q], f32, tag="sc", name=f"wsc_{wi}")
            nc.tensor.matmul(
                wsc[:, 0:512], wsrc[:, 0:128], wsrc[:],
                start=True, stop=True, skip_group_check=True,
            )
        emit_loads(1)

        # Cross-pass software pipeline: the last two PV pairs (and the final
        # acc chunk's evacuation) of pass p are emitted right after the first
        # QK pair of the next pass, so the PE never drains at pass/head
        # boundaries.
        pending_tail = [None]

        def _flush_tail():
            if pending_tail[0] is not None:
                pending_tail[0]()
                pending_tail[0] = None

        for h in range(n_heads):
            qt2, kt2, vx = qts[h], kts[h], vxs[h]
            vx_v = vx.rearrange("p (t c) -> p t c", c=65)
            kt2_v = kt2.rearrange("p (t c) -> p t c", c=128)

            p_order = (
                list(range(n_pass)) if h < n_heads - 1
                else list(reversed(range(n_pass)))
            )
            for pi, p in enumerate(p_order):
                if h + 2 <= n_heads - 1 and pi == 1:
                    emit_loads(h + 2)
                q0 = p * pass_q
                kmax = (p + 1) * ktiles_per_pass
                # per-512-chunk accumulators; chunk c's chain ends at k-tile
                # last_w[c], whose PV carries the stop flag -> early evac
                accs = [
                    accApool.tile([65, CHUNK], f32, name="acc", tag="acc")
                    for c in range(n_chunks)
                ]
                last_w = [
                    min(kmax, (q0 + (c + 1) * CHUNK) // 128) - 1
                    for c in range(n_chunks)
                ]
                pv_queue = []

                def _evac(c, h=h, p=p, q0=q0, accs=accs):
                    """Evacuate acc chunk c (fp32 PSUM -> fp16 SBUF) + DMA."""
                    osb = osbpool.tile([65, CHUNK], f16, name="osb", tag="osb")
                    a0 = q0 + c * CHUNK
                    if c % 2 == 0:
                        nc.scalar.copy(osb[:], accs[c][:])
                    else:
                        nc.vector.tensor_copy(osb[:], accs[c][:])
                    nc.sync.dma_start(ot_d[h, :, a0 : a0 + CHUNK], osb[:])

                def _evac_piece(i, lo, hi, h=h, p=p, q0=q0, accs=accs):
                    """Final-pass fine-grained evac of acc cols [lo,hi) as the
                    per-column accumulation chains end; alternate engines and
                    DMA queues so the last piece's path is minimal."""
                    osb = osbpool.tile([65, CHUNK], f16, name="osb", tag="osb")
                    c, w = lo // CHUNK, hi - lo
                    cc = lo % CHUNK
                    if i % 2 == 0:
                        nc.scalar.copy(osb[:, 0:w], accs[c][:, cc : cc + w])
                    else:
                        nc.vector.tensor_copy(osb[:, 0:w], accs[c][:, cc : cc + w])
                    dma_eng = nc.sync if i % 2 == 0 else nc.scalar
                    dma_eng.dma_start(ot_d[h, :, q0 + lo : q0 + hi], osb[:, 0:w])

                def _emit_pv(entries, accs=accs, last_w=last_w, q0=q0, vx_v=vx_v,
                             evac=_evac, evacp=_evac_piece, kmax=kmax, final=False):
                    for (k, at, qlo) in entries:
                        for (c, w) in _chunks(qlo - q0, pass_q):
                            co = c - (qlo - q0)
                            ci, cc = c // CHUNK, c % CHUNK
                            nc.tensor.matmul(
                                accs[ci][0:65, cc : cc + w],
                                vx_v[:, k, :],
                                at[:, co : co + w],
                                start=(k == 0),
                                stop=(k == last_w[ci]),
                                skip_group_check=True,
                            )
                        if final and k >= kmax - 4:
                            lo = max(CHUNK, 128 * k - q0)
                            hi = min(pass_q, 128 * (k + 1) - q0)
                            if hi > lo:
                                evacp(k, lo, hi)
                        for ci in range(len(accs)):
                            if k == last_w[ci] and ci < len(accs) - 1:
                                evac(ci)

                # exp engine assignment: balance ScalarE (0.833ns/col+185)
                # vs DVE (1.04ns/col+125): DVE takes odd k-tiles except the
                # two largest odd spans per pass go to ScalarE.
                for kp in range(0, kmax, 2):
                    pair = [k for k in (kp, kp + 1) if k < kmax]
                    scs, spans, qlos = {}, {}, {}
                    for k in pair:
                        qlos[k] = max(q0, 128 * k)
                        spans[k] = q0 + pass_q - qlos[k]
                        scs[k] = scpool.tile(
                            [128, pass_q], f32, tag="sc", name=f"sc_{h}_{p}_{k}"
                        )
                    for k in pair:
                        for (c, w) in _chunks(0, spans[k]):
                            nc.tensor.matmul(
                                scs[k][:, c : c + w],
                                kt2_v[:, k],
                                qt2[:, qlos[k] + c : qlos[k] + c + w],
                                start=True,
                                stop=True,
                                skip_group_check=True,
                            )
                    if kp == 0:
                        _flush_tail()
                    cur = []
                    for k in pair:
                        span, qlo = spans[k], qlos[k]
                        at = atpool.tile([128, pass_q], f16)
                        if k % 2 == 0:
                            nc.scalar.activation(
                                at[:, 0:span], scs[k][:, 0:span], Exp,
                                bias=c_beta[:, 0:1], scale=0.0625,
                            )
                        else:
                            nc.vector._custom_dve(
                                exp_op,
                                out=at[:, 0:span],
                                in0=scs[k][:, 0:span],
                                s0=EXP_C0, s1=EXP_C1, imm2=EXP_C2,
                            )
                        if 128 * k >= q0:
                            # zero the masked upper part of the diagonal block
                            nc.gpsimd.tensor_mul(at[:, 0:128], at[:, 0:128], c_tri[:])
                        cur.append((k, at, qlo))
                    pv_queue.append(cur)
                    if len(pv_queue) > 3:
                        _emit_pv(pv_queue.pop(0))
                leftovers = list(pv_queue)
                is_final = h == n_heads - 1 and pi == n_pass - 1

                def _tail(leftovers=leftovers, emit=_emit_pv, evac=_evac,
                          nch=n_chunks, final=is_final):
                    for entries in leftovers:
                        emit(entries, final=final)
                    if not final:
                        evac(nch - 1)

                if is_final:
                    _tail()
                else:
                    pending_tail[0] = _tail
        _flush_tail()


def _make_consts():
    kk, qq = np.meshgrid(np.arange(128), np.arange(128), indexing="ij")
    tri = (kk <= qq).astype(np.float16)  # keep-mask for the diagonal block
    return tri


_NC_CACHE = {}


def _build_nc(n_heads=HEADS_PER_CORE, s=S, pass_q=PASS_Q):
    key = (n_heads, s, pass_q)
    if key in _NC_CACHE:
        return _NC_CACHE[key]
    import concourse.tile as tile
    from concourse import bacc, mybir

    nc = bacc.Bacc(
        "TRN2", target_bir_lowering=False, debug=False, enable_asserts=False
    )
    f32 = mybir.dt.float32
    f16 = mybir.dt.float16
    ins = {
        "qt": nc.dram_tensor("qt", [n_heads, 128, s], f16, kind="ExternalInput").ap(),
        "kt": nc.dram_tensor("kt", [n_heads, 128, s], f16, kind="ExternalInput").ap(),
        "v": nc.dram_tensor("v", [n_heads, s, D + 1], f16, kind="ExternalInput").ap(),
        "ctri": nc.dram_tensor("ctri", [128, 128], f16, kind="ExternalInput").ap(),
        "cbeta": nc.dram_tensor("cbeta", [128, 1], f32, kind="ExternalInput").ap(),
    }
    outs = {
        "ot": nc.dram_tensor("ot", [n_heads, 65, s], f16, kind="ExternalOutput").ap(),
    }
    with tile.TileContext(nc) as tc:
        build_attention(tc, outs, ins, n_heads=n_heads, s=s, pass_q=pass_q)
    nc.compile()
    _NC_CACHE[key] = nc
    return nc


def kernel(Q, K, V, mask, trace=False):
    """Full-input entry point: shards over 8 NeuronCores, returns full output."""
    from concourse.bass_utils import run_bass_kernel_spmd

    nc = _build_nc()
    tri = _make_consts()

    Qf = np.ascontiguousarray(
        Q.reshape(B * H, S, D).transpose(0, 2, 1), dtype=np.float16
    )
    Kf = np.ascontiguousarray(
        K.reshape(B * H, S, D).transpose(0, 2, 1), dtype=np.float16
    )
    # duplicate into both partition halves host-side (one DMA instr per load)
    Qf = np.ascontiguousarray(np.concatenate([Qf, Qf], axis=1))
    Kf = np.ascontiguousarray(np.concatenate([Kf, Kf], axis=1))
    Vf = np.concatenate(
        [
            V.reshape(B * H, S, D).astype(np.float16),
            np.ones((B * H, S, 1), dtype=np.float16),
        ],
        axis=-1,
    )

    in_maps = []
    for c in range(N_CORES):
        sl = slice(c * HEADS_PER_CORE, (c + 1) * HEADS_PER_CORE)
        in_maps.append(
            {
                "qt": Qf[sl],
                "kt": Kf[sl],
                "v": Vf[sl],
                "ctri": tri,
                "cbeta": np.full((128, 1), EXP_BETA, dtype=np.float32),
            }
        )

    res = run_bass_kernel_spmd(nc, in_maps, core_ids=list(range(N_CORES)), trace=trace)
    ot = np.concatenate(
        [res.results[c]["ot"].astype(np.float32) for c in range(N_CORES)], axis=0
    )
    # ot: [B*H, 65, S] -- rows 0..63 are out^T columns, row 64 the rowsum.
    out = (ot[:, :64, :] / ot[:, 64:65, :]).transpose(0, 2, 1)
    out = out.reshape(B, H, S, D)
    kernel.last_results = res
    return np.ascontiguousarray(out, dtype=np.float32)
